# revision 1
# baseline (speedup 1.0000x reference)
"""GAT (graph attention) kernel for Trainium2, 8-core SPMD — one head per core.

Reference computation (per head k):
    h = x @ W_k.T + b_k                       # (N, F)
    left[n]  = h[n] . a_left_k ; right[m] = h[m] . a_right_k
    e[n, m]  = leaky_relu(left[n] + right[m], 0.2)
    a        = softmax_m(where(mask[n, m], e, -1e9))
    out_k    = elu(a @ h)                      # (N, F)
Full output = concat_k(out_k)  -> (N, K*F)

Device strategy (per core, attention tiles are [m(partition), n(free)]):
    - hijacked ACT `Exp` table computes exp(leaky_relu(x, 0.2)) in one pass
      (negative-x spline buckets refit to exp(0.2x); positive side untouched,
      so plain exp(v) for v<=0 is recovered with scale=5).
    - project h_T[f, n] = W_k.T.T @ x.T on PE (fp32), bias-add into SBUF
    - left/right via one PE matmul with lhsT = [a_left | a_right]
    - h in [m, f] chunks (lhsT for aggregation) via PE transposes -> bf16
    - main loop over (n-half, m-chunk):
        em  = exp(leaky(left[n] + right[m]))   (one ScalarE inst, bias=right)
        em *= mask                             (VectorE bf16 tensor_tensor, 2x)
        outT[f, n] += h_chunk.T @ em ; sums[n] += ones.T @ em   (PE, PSUM)
    - epilogue: rs = 1/sums, u = outT * rs, elu (exp via scale=5), store [f, n]
    - host transposes out to [n, f] and concatenates heads.

No row-max subtraction is needed: z in [-13, 13] for these input scales.
Masked entries contribute exactly 0 (mask multiply happens after exp).
"""

import json
import os
import shutil
import tempfile

import numpy as np

import concourse.bass as bass
import concourse.tile as tile
from concourse import bacc, mybir
from concourse.bass_utils import run_bass_kernel_spmd
from concourse.masks import make_identity

N_NODES = 4096
F_IN = 512
K_HEADS = 8
F_OUT = 128
NEG_SLOPE = 0.2
N_CORES = 8

f32 = mybir.dt.float32
bf16 = mybir.dt.bfloat16


# --------------------------------------------------------------------------- #
# activation-table hack: make `exp` compute exp(leaky_relu(x, 0.2))
# --------------------------------------------------------------------------- #
def _make_hacked_act_dir(dst):
    from neuronxcc.driver.Job import Job
    from neuronxcc.driver.jobs.support.FindActInfo import findActInfoFile

    src = os.path.dirname(findActInfoFile(Job.getPackageDir(), "gen3"))
    os.makedirs(dst, exist_ok=True)
    for fn in os.listdir(src):
        shutil.copy(os.path.join(src, fn), os.path.join(dst, fn))

    info = json.load(open(os.path.join(dst, "act_info.json")))
    for s in info["act_func_sets"]:
        if "exp" not in s["act"]:
            continue
        prof = json.load(open(os.path.join(dst, s["profile_json"])))
        start = prof["func_to_bkt_start_idx"]["exp"]
        starts = sorted(prof["func_to_bkt_start_idx"].values())
        ends = [e for e in starts if e > start]
        end = ends[0] if ends else prof["bkt_entry_cnt"]

        path = os.path.join(dst, s["bkt_bin"])
        b = np.fromfile(path, dtype=np.float32).reshape(-1, 8).copy()
        sl = b[start:end]
        neg = sl[:, 4] < 0.0
        x0 = sl[neg, 4].astype(np.float64)
        g = np.exp(NEG_SLOPE * x0)
        sl[neg, 0] = g
        sl[neg, 1] = NEG_SLOPE * g
        sl[neg, 2] = NEG_SLOPE**2 * g / 2.0
        sl[neg, 3] = NEG_SLOPE**3 * g / 6.0
        b[start:end] = sl
        b.tofile(path)
    return os.path.join(dst, "act_info.json")


_ACT_DIR = None


def setup_act_tables():
    global _ACT_DIR
    if _ACT_DIR is None:
        d = os.path.join(tempfile.gettempdir(), "gat_act_tables")
        _ACT_DIR = _make_hacked_act_dir(d)
    os.environ["BASS_ACT_ROOT_JSON_PATH"] = _ACT_DIR
    return _ACT_DIR


# --------------------------------------------------------------------------- #
# bass program
# --------------------------------------------------------------------------- #
def build(n_nodes=N_NODES, n_tile=2048, num_devices=N_CORES, timing_mode=False, repeat=1):
    """One head per core. Returns compiled Bacc module.

    timing_mode: large inputs/outputs become Internal DRAM (no host traffic);
    the whole compute body is emitted `repeat` times so device time dominates
    dispatch overhead."""
    setup_act_tables()

    n = n_nodes
    mc_cnt = n // 128          # m-chunks
    halves = n // n_tile       # n-range splits
    cseg = F_IN // 128         # contraction chunks for the projection
    nseg = min(512, n)         # matmul moving-operand segment (PSUM bank)
    tseg = min(512, n_tile)

    nc = bacc.Bacc("TRN2", target_bir_lowering=False, debug=False, num_devices=num_devices)

    big_kind = "Internal" if timing_mode else "ExternalInput"
    xT_d = nc.dram_tensor("xT", [F_IN, n], f32, kind=big_kind).ap()
    wkT_d = nc.dram_tensor("wkT", [F_IN, F_OUT], f32, kind="ExternalInput").ap()
    bk_d = nc.dram_tensor("bk", [F_OUT, 1], f32, kind="ExternalInput").ap()
    alr_d = nc.dram_tensor("alr", [F_OUT, 2], f32, kind="ExternalInput").ap()
    maskT_d = nc.dram_tensor("maskT", [n, n], bf16, kind=big_kind).ap()
    out_kind = "Internal" if timing_mode else "ExternalOutput"
    out_d = nc.dram_tensor("out", [F_OUT, n], f32, kind=out_kind).ap()
    sink_d = None
    if timing_mode:
        sink_d = nc.dram_tensor("sink", [1, 128], f32, kind="ExternalOutput").ap()

    lr_dram = nc.dram_tensor("lr_scratch", [2, n], f32, kind="Internal")
    sums_dram = nc.dram_tensor("sums_scratch", [halves, n_tile], f32, kind="Internal")
    rs_dram = nc.dram_tensor("rs_scratch", [halves, n_tile], f32, kind="Internal")

    def dram_ap(handle, offset, pattern):
        return bass.AP(tensor=handle.ap().tensor, offset=offset, ap=pattern)

    with tile.TileContext(nc) as tc:
        with (
            tc.tile_pool(name="consts", bufs=1) as consts,
            tc.tile_pool(name="work", bufs=3) as work,
            tc.tile_pool(name="epi", bufs=1) as epi,
        ):
            if timing_mode:
                # fill the Internal inputs on-device: x = 0, mask = 1
                fz = consts.tile([128, n], f32, tag="bigbuf")
                nc.vector.memset(fz, 0.0)
                for c in range(cseg):
                    nc.sync.dma_start(out=xT_d[c * 128 : (c + 1) * 128, :], in_=fz)
                fo = consts.tile([128, n], bf16, tag="fo")
                nc.vector.memset(fo, 1.0)
                for r in range(n // 128):
                    nc.sync.dma_start(out=maskT_d[r * 128 : (r + 1) * 128, :], in_=fo)

            emitted_o_sb = [None]
            for _rep in range(repeat):
              # ---------------- phase 0: load constants ---------------- #
              xT_sb = consts.tile([128, cseg, n], f32, tag="bigbuf")
              for c in range(cseg):
                  nc.sync.dma_start(out=xT_sb[:, c, :], in_=xT_d[c * 128 : (c + 1) * 128, :])
              wkT_sb = consts.tile([128, cseg, F_OUT], f32)
              for c in range(cseg):
                  nc.sync.dma_start(out=wkT_sb[:, c, :], in_=wkT_d[c * 128 : (c + 1) * 128, :])
              bk_sb = consts.tile([128, 1], f32)
              nc.sync.dma_start(out=bk_sb, in_=bk_d)
              alr_sb = consts.tile([128, 2], f32)
              nc.sync.dma_start(out=alr_sb, in_=alr_d)
              identity = consts.tile([128, 128], f32)
              make_identity(nc, identity)
              ones_sb = consts.tile([128, 1], bf16)
              nc.vector.memset(ones_sb, 1.0)

              # ---------------- phase 1: h_T = (W_k x.T) + b ---------------- #
              hT_sb = consts.tile([128, n], f32)
              with tc.tile_pool(name="psA", bufs=1, space="PSUM") as psA:
                  hT_ps = psA.tile([128, n], f32, tag="big")
                  for c in range(cseg):
                      for s in range(n // nseg):
                          nc.tensor.matmul(
                              hT_ps[:, s * nseg : (s + 1) * nseg],
                              lhsT=wkT_sb[:, c, :],
                              rhs=xT_sb[:, c, s * nseg : (s + 1) * nseg],
                              start=(c == 0),
                              stop=(c == cseg - 1),
                          )
                  nc.vector.tensor_scalar_add(out=hT_sb, in0=hT_ps, scalar1=bk_sb)

                  # left/right: lr[2, n] = [a_l | a_r].T @ h_T
                  lr_ps = psA.tile([2, n], f32, tag="big")
                  for s in range(n // nseg):
                      nc.tensor.matmul(
                          lr_ps[:, s * nseg : (s + 1) * nseg],
                          lhsT=alr_sb,
                          rhs=hT_sb[:, s * nseg : (s + 1) * nseg],
                          start=True,
                          stop=True,
                      )
                  lr_sb = consts.tile([2, n], f32, tag="bigbuf")
                  nc.vector.tensor_copy(out=lr_sb, in_=lr_ps)
                  nc.sync.dma_start(out=lr_dram.ap(), in_=lr_sb)

              # broadcasts / reshapes of left & right (via DRAM roundtrip)
              left_bc = consts.tile([128, n], f32)
              nc.sync.dma_start(out=left_bc, in_=dram_ap(lr_dram, 0, [[0, 128], [1, n]]))
              right_sc = consts.tile([128, mc_cnt], f32)
              nc.sync.dma_start(
                  out=right_sc, in_=dram_ap(lr_dram, n, [[1, 128], [128, mc_cnt]])
              )

              # ---------------- phase 2: h in [m, f] chunks (bf16) ---------------- #
              h_mf = consts.tile([128, mc_cnt, F_OUT], bf16)
              with tc.tile_pool(name="psB", bufs=4, space="PSUM") as psB:
                  for j in range(mc_cnt):
                      tr_ps = psB.tile([128, 128], f32, tag="tr")
                      nc.tensor.transpose(tr_ps, hT_sb[:, j * 128 : (j + 1) * 128], identity)
                      nc.vector.tensor_copy(out=h_mf[:, j, :], in_=tr_ps)

              # ---------------- phase 3: main attention loop ---------------- #
              with tc.tile_pool(name="psC", bufs=1, space="PSUM") as psC:
                  for half in range(halves):
                      n0 = half * n_tile
                      outT_ps = psC.tile([128, n_tile], f32, tag="outT")
                      sums_ps = psC.tile([1, n_tile], f32, tag="sums")

                      for mc in range(mc_cnt):
                          mask_sb = work.tile([128, n_tile], bf16, tag="mask")
                          nc.sync.dma_start(
                              out=mask_sb,
                              in_=maskT_d[mc * 128 : (mc + 1) * 128, n0 : n0 + n_tile],
                          )
                          # em = exp(leaky(left + right)) in ONE ScalarE pass
                          # (hacked Exp table; bias = per-partition right)
                          em_sb = work.tile([128, n_tile], bf16, tag="em")
                          nc.scalar.activation(
                              out=em_sb,
                              in_=left_bc[:, n0 : n0 + n_tile],
                              func=mybir.ActivationFunctionType.Exp,
                              bias=right_sc[:, mc : mc + 1],
                              scale=1.0,
                          )
                          # em *= mask  (bf16 tensor_tensor, 2x mode, in place)
                          nc.vector.tensor_tensor(
                              out=em_sb, in0=em_sb, in1=mask_sb, op=mybir.AluOpType.mult
                          )
                          first, last = mc == 0, mc == mc_cnt - 1
                          for s in range(n_tile // tseg):
                              nc.tensor.matmul(
                                  outT_ps[:, s * tseg : (s + 1) * tseg],
                                  lhsT=h_mf[:, mc, :],
                                  rhs=em_sb[:, s * tseg : (s + 1) * tseg],
                                  start=first,
                                  stop=last,
                              )
                          for s in range(n_tile // tseg):
                              nc.tensor.matmul(
                                  sums_ps[:, s * tseg : (s + 1) * tseg],
                                  lhsT=ones_sb,
                                  rhs=em_sb[:, s * tseg : (s + 1) * tseg],
                                  start=first,
                                  stop=last,
                              )

                      # ---- epilogue for this half ---- #
                      sums_sb = epi.tile([1, n_tile], f32, tag="sums_sb")
                      nc.vector.tensor_copy(out=sums_sb, in_=sums_ps)
                      nc.sync.dma_start(
                          out=sums_dram.ap()[half : half + 1, :], in_=sums_sb
                      )
                      sums_sc = epi.tile([128, n_tile // 128], f32, tag="sums_sc")
                      nc.sync.dma_start(
                          out=sums_sc,
                          in_=dram_ap(
                              sums_dram, half * n_tile, [[1, 128], [128, n_tile // 128]]
                          ),
                      )
                      rs_sc = epi.tile([128, n_tile // 128], f32, tag="rs_sc")
                      nc.vector.reciprocal(out=rs_sc, in_=sums_sc)
                      nc.sync.dma_start(
                          out=dram_ap(
                              rs_dram, half * n_tile, [[1, 128], [128, n_tile // 128]]
                          ),
                          in_=rs_sc,
                      )
                      rs_bc = epi.tile([128, n_tile], f32, tag="rs_bc")
                      nc.sync.dma_start(
                          out=rs_bc,
                          in_=dram_ap(rs_dram, half * n_tile, [[0, 128], [1, n_tile]]),
                      )
                      # u = outT * rs ; elu(u) = max(u, exp(min(u, 0)) - 1)
                      # (exp of a negative via hacked table: scale=5 recovers exp)
                      u_sb = epi.tile([128, n_tile], f32, tag="u")
                      nc.vector.tensor_tensor(
                          out=u_sb, in0=outT_ps, in1=rs_bc, op=mybir.AluOpType.mult
                      )
                      t_sb = epi.tile([128, n_tile], f32, tag="t")
                      nc.vector.tensor_scalar_min(out=t_sb, in0=u_sb, scalar1=0.0)
                      nc.scalar.activation(
                          out=t_sb,
                          in_=t_sb,
                          func=mybir.ActivationFunctionType.Exp,
                          scale=5.0,
                      )
                      o_sb = epi.tile([128, n_tile], f32, tag="o")
                      nc.vector.scalar_tensor_tensor(
                          out=o_sb,
                          in0=t_sb,
                          scalar=-1.0,
                          in1=u_sb,
                          op0=mybir.AluOpType.add,
                          op1=mybir.AluOpType.max,
                      )
                      nc.sync.dma_start(out=out_d[:, n0 : n0 + n_tile], in_=o_sb)
                      emitted_o_sb[0] = o_sb

            if timing_mode and sink_d is not None:
                nc.sync.dma_start(out=sink_d, in_=emitted_o_sb[0][0:1, 0:128])

    nc.compile()
    return nc


# --------------------------------------------------------------------------- #
# host entry point
# --------------------------------------------------------------------------- #
_NC_CACHE = {}


def _get_nc():
    key = (N_NODES, 2048)
    if key not in _NC_CACHE:
        _NC_CACHE[key] = build(N_NODES, 2048, N_CORES)
    return _NC_CACHE[key]


def make_in_maps(x, mask, W, b, a_left, a_right):
    import ml_dtypes

    xT = np.ascontiguousarray(x.T, dtype=np.float32)
    maskT = np.ascontiguousarray(mask.T).astype(ml_dtypes.bfloat16)
    in_maps = []
    for k in range(K_HEADS):
        Wk = W[k * F_OUT : (k + 1) * F_OUT, :]
        in_maps.append(
            {
                "xT": xT,
                "wkT": np.ascontiguousarray(Wk.T, dtype=np.float32),
                "bk": np.ascontiguousarray(
                    b[k * F_OUT : (k + 1) * F_OUT].reshape(F_OUT, 1), dtype=np.float32
                ),
                "alr": np.ascontiguousarray(
                    np.stack([a_left[k], a_right[k]], axis=1), dtype=np.float32
                ),
                "maskT": maskT,
            }
        )
    return in_maps


def kernel(x, mask, W, b, a_left, a_right):
    x = np.asarray(x)
    mask = np.asarray(mask)
    W = np.asarray(W)
    b = np.asarray(b)
    a_left = np.asarray(a_left)
    a_right = np.asarray(a_right)
    nc = _get_nc()
    in_maps = make_in_maps(x, mask, W, b, a_left, a_right)
    res = run_bass_kernel_spmd(nc, in_maps, core_ids=list(range(N_CORES)))
    outs = [np.ascontiguousarray(res.results[k]["out"].T) for k in range(K_HEADS)]
    return np.concatenate(outs, axis=1)


if __name__ == "__main__":
    import reference as R

    inputs = {k: np.asarray(v) for k, v in R.setup_inputs().items()}
    expected = np.asarray(R.reference(**R.setup_inputs()))
    got = kernel(**inputs)
    aerr = np.abs(got - expected)
    scale = np.abs(expected).max()
    print(f"absmax err {aerr.max():.3e}  scale {scale:.3f}  rel {aerr.max() / scale:.3e}")

